# revision 2
# baseline (speedup 1.0000x reference)
"""Trainium2 Bass kernel for nn_Criterion_67954972557706 (retrieval_knn).

Computes, per batch element: brute-force K=1 NN from N cloth vertices to F
obstacle-face centroids (argmin over an [N, F] squared-distance matrix), then
a signed-distance interpenetration loss against the matched face plane.

Sharding: 8 cores = 2 batches x 4 row-shards of N=16384 (4096 rows/core).
Each core scans all F=16384 faces for its rows.

Device kernel (per core, per 128-row tile):
  PE     : scores[128, F] = 2*x.c - |c|^2  (K=4 matmul, argmax == argmin d2)
  DVE    : per-32-wide mini-chunk max (tensor_reduce) -> cm[128, F/32]
  ACT    : copy scores PSUM -> SBUF (gather source)
  DVE    : max8 + max_index over cm -> winning mini-chunk c* per row
  GPSIMD : ap_gather: for each 16-partition group, gather the group's 16
           winning mini-chunks (32 f32 each) from every partition's row
  DVE    : max_index over gathered [128, 512] -> position of global max
Host combines c*, i* -> nn_idx, then does the tiny gather + loss epilogue
in float32 numpy (0.03% of the FLOPs).
"""

import numpy as np

import concourse.bass as bass
import concourse.bacc as bacc
import concourse.mybir as mybir
import concourse.tile as tile
from concourse.bass_utils import run_bass_kernel_spmd

EPS = 1e-3
WEIGHT_START = 1e-3
WEIGHT_MAX = 5e3
START_RAMPUP = 50000
N_RAMPUP = 100000

MINI = 32          # mini-chunk width for the two-level argmax
CHUNK = 2048       # psum tile width (4 banks); 4 matmuls of 512 per chunk
N_CORES = 8

_compiled_cache = {}


def build_knn_kernel(row_tiles: int, F: int):
    """Build + compile the per-core KNN argmax kernel.

    Inputs (DRAM):
      xt4  [row_tiles, 4, 128] f32 : per row-tile lhsT; rows 0-2 = x coords,
                                     row 3 = 1.0
      crhs [4, F] f32              : rows 0-2 = 2*centroid coords, row 3 = -|c|^2
    Outputs:
      cidx [128, row_tiles] u32    : winning mini-chunk per row
      widx [128, row_tiles] u32    : position of the max within the gathered
                                     [16, MINI] block (j = widx % MINI)
    """
    n_chunks = F // CHUNK
    cm_w = F // MINI
    cm_per_chunk = CHUNK // MINI
    dt = mybir.dt

    nc = bacc.Bacc("TRN2", target_bir_lowering=False)

    xt4 = nc.declare_dram_parameter("xt4", [row_tiles, 4, 128], dt.float32, isOutput=False)
    crhs = nc.declare_dram_parameter("crhs", [4, F], dt.float32, isOutput=False)
    cidx_d = nc.declare_dram_parameter("cidx", [128, row_tiles], dt.uint32, isOutput=True)
    widx_d = nc.declare_dram_parameter("widx", [128, row_tiles], dt.uint32, isOutput=True)

    with tile.TileContext(nc) as tc:
        with (
            tc.tile_pool(name="lhs_pool", bufs=3) as lhs_pool,
            tc.tile_pool(name="rhs_pool", bufs=4) as rhs_pool,
            tc.tile_pool(name="psum_pool", bufs=2, space="PSUM") as psum_pool,
            tc.tile_pool(name="scores_pool", bufs=2) as scores_pool,
            tc.tile_pool(name="cm_pool", bufs=2) as cm_pool,
            tc.tile_pool(name="small_pool", bufs=3) as small_pool,
            tc.tile_pool(name="g_pool", bufs=2) as g_pool,
            tc.tile_pool(name="out_pool", bufs=1) as out_pool,
        ):
            cidx_t = out_pool.tile([128, row_tiles], dt.uint32, name="cidx_t")
            widx_t = out_pool.tile([128, row_tiles], dt.uint32, name="widx_t")

            for t in range(row_tiles):
                lhs = lhs_pool.tile([4, 128], dt.float32, name="lhs")
                nc.sync.dma_start(out=lhs[:], in_=xt4[t])

                sc = scores_pool.tile([128, F], dt.float32, name="sc")
                cm = cm_pool.tile([128, cm_w], dt.float32, name="cm")

                for q in range(n_chunks):
                    rhs = rhs_pool.tile([4, CHUNK], dt.float32, name="rhs")
                    nc.sync.dma_start(out=rhs[:], in_=crhs[:, q * CHUNK:(q + 1) * CHUNK])
                    ps = psum_pool.tile([128, CHUNK], dt.float32, name="ps")
                    for m in range(CHUNK // 512):
                        nc.tensor.matmul(
                            ps[:, m * 512:(m + 1) * 512],
                            lhs[:],
                            rhs[:, m * 512:(m + 1) * 512],
                            start=True,
                            stop=True,
                        )
                    nc.vector.tensor_reduce(
                        cm[:, q * cm_per_chunk:(q + 1) * cm_per_chunk],
                        ps.rearrange("p (a b) -> p a b", b=MINI),
                        axis=mybir.AxisListType.X,
                        op=mybir.AluOpType.max,
                    )
                    nc.scalar.copy(sc[:, q * CHUNK:(q + 1) * CHUNK], ps[:])

                gm8 = small_pool.tile([128, 8], dt.float32, name="gm8")
                nc.vector.max(gm8[:], cm[:])
                c8 = small_pool.tile([128, 8], dt.uint32, name="c8")
                nc.vector.max_index(c8[:], gm8[:], cm[:])
                i16 = small_pool.tile([128, 1], dt.int16, name="i16")
                nc.vector.tensor_copy(i16[:], c8[:, 0:1])

                g = g_pool.tile([128, 16 * MINI], dt.float32, name="g")
                nc.gpsimd.ap_gather(
                    g.rearrange("p (i d) -> p i d", d=MINI),
                    sc.rearrange("p (n d) -> p n d", d=MINI),
                    i16[:],
                    channels=128,
                    num_elems=cm_w,
                    d=MINI,
                    num_idxs=16,
                )
                w8 = small_pool.tile([128, 8], dt.uint32, name="w8")
                nc.vector.max_index(w8[:], gm8[:], g[:])

                nc.vector.tensor_copy(cidx_t[:, t:t + 1], c8[:, 0:1])
                nc.vector.tensor_copy(widx_t[:, t:t + 1], w8[:, 0:1])

            nc.sync.dma_start(out=cidx_d[:], in_=cidx_t[:])
            nc.sync.dma_start(out=widx_d[:], in_=widx_t[:])

    nc.compile()
    return nc


def _get_compiled(row_tiles: int, F: int):
    key = (row_tiles, F)
    if key not in _compiled_cache:
        _compiled_cache[key] = build_knn_kernel(row_tiles, F)
    return _compiled_cache[key]


def _face_tables(obs_pos: np.ndarray, faces: np.ndarray):
    """face centroids [F,3] for one batch, float32."""
    v = obs_pos[faces]                     # [F, 3, 3]
    return v.mean(axis=1, dtype=np.float32)


def _prep_core_inputs(x: np.ndarray, face_curr: np.ndarray, row_tiles: int):
    """x: [rows, 3] cloth slice; face_curr: [F, 3]. Returns xt4, crhs."""
    rows = x.shape[0]
    assert rows == row_tiles * 128
    xt4 = np.empty((row_tiles, 4, 128), dtype=np.float32)
    xr = x.reshape(row_tiles, 128, 3)
    xt4[:, :3, :] = np.transpose(xr, (0, 2, 1))
    xt4[:, 3, :] = 1.0
    c2 = np.sum(face_curr.astype(np.float32) ** 2, axis=1, dtype=np.float32)
    crhs = np.empty((4, face_curr.shape[0]), dtype=np.float32)
    crhs[:3] = 2.0 * face_curr.T
    crhs[3] = -c2
    return xt4, crhs


class _Runner:
    """Persistent jitted PJRT runner for the SPMD bass kernel.

    Mirrors bass2jax.run_bass_via_pjrt but keeps the jitted callable (and the
    device-resident inputs) alive so repeated invocations measure device time
    rather than jit re-trace + host transfer.
    """

    def __init__(self, nc, n_cores: int):
        import jax
        from concourse import bass2jax
        from jax.experimental.shard_map import shard_map
        from jax.sharding import Mesh, PartitionSpec

        bass2jax.install_neuronx_cc_hook()
        self.jax = jax
        self.n_cores = n_cores

        partition_name = (
            nc.partition_id_tensor.name if nc.partition_id_tensor else None
        )
        in_names, out_names, out_avals, zero_outs = [], [], [], []
        for alloc in nc.m.functions[0].allocations:
            if not isinstance(alloc, mybir.MemoryLocationSet):
                continue
            name = alloc.memorylocations[0].name
            if alloc.kind == "ExternalInput":
                if name != partition_name:
                    in_names.append(name)
            elif alloc.kind == "ExternalOutput":
                shape = tuple(alloc.tensor_shape)
                dtype = mybir.dt.np(alloc.dtype)
                out_names.append(name)
                out_avals.append(jax.core.ShapedArray(shape, dtype))
                zero_outs.append(np.zeros(shape, dtype))
        self.in_names = in_names
        self.out_names = out_names
        self.out_avals = out_avals
        self.zero_outs = zero_outs
        n_params = len(in_names)
        n_outs = len(out_avals)
        all_names = in_names + out_names
        if partition_name is not None:
            all_names.append(partition_name)

        def _body(*args):
            operands = list(args)
            if partition_name is not None:
                operands.append(bass2jax.partition_id_tensor())
            outs = bass2jax._bass_exec_p.bind(
                *operands,
                out_avals=tuple(out_avals),
                in_names=tuple(all_names),
                out_names=tuple(out_names),
                lowering_input_output_aliases=(),
                sim_require_finite=True,
                sim_require_nnan=True,
                nc=nc,
            )
            return tuple(outs)

        devices = jax.devices()[:n_cores]
        self.mesh = Mesh(np.asarray(devices), ("core",))
        in_specs = (PartitionSpec("core"),) * (n_params + n_outs)
        out_specs = (PartitionSpec("core"),) * n_outs
        # no donation: lets us re-run with the same device buffers
        self.fn = jax.jit(
            shard_map(_body, mesh=self.mesh, in_specs=in_specs,
                      out_specs=out_specs, check_rep=False),
            keep_unused=True,
        )

    def place(self, in_maps):
        """device_put concatenated inputs; returns arg list."""
        import jax
        from jax.sharding import NamedSharding, PartitionSpec

        n = self.n_cores
        sh = NamedSharding(self.mesh, PartitionSpec("core"))
        args = []
        for name in self.in_names:
            arr = np.concatenate([np.asarray(m[name]) for m in in_maps], axis=0)
            args.append(jax.device_put(arr, sh))
        for z in self.zero_outs:
            zz = np.zeros((n * z.shape[0], *z.shape[1:]), z.dtype)
            args.append(jax.device_put(zz, sh))
        return args

    def run(self, args):
        outs = self.fn(*args)
        self.jax.block_until_ready(outs)
        return outs

    def results(self, outs):
        n = self.n_cores
        return [
            {
                name: np.asarray(outs[i]).reshape(n, *self.out_avals[i].shape)[c]
                for i, name in enumerate(self.out_names)
            }
            for c in range(n)
        ]


_runner_cache = {}


def _get_runner(row_tiles, F, n_cores):
    key = (row_tiles, F, n_cores)
    if key not in _runner_cache:
        nc = _get_compiled(row_tiles, F)
        _runner_cache[key] = _Runner(nc, n_cores)
    return _runner_cache[key]


class _RunResult:
    def __init__(self, results, exec_time_ns=None):
        self.results = results
        self.exec_time_ns = exec_time_ns


def _run_device(in_maps, row_tiles, F, trace=False):
    runner = _get_runner(row_tiles, F, len(in_maps))
    args = runner.place(in_maps)
    outs = runner.run(args)
    return _RunResult(runner.results(outs))


def compute_nn_idx(cloth_curr_pos, obstacle_curr_pos, obstacle_faces, trace=False):
    """Device round-trip: returns nn_idx [B, N] int64 (+ profile results)."""
    B, N, _ = cloth_curr_pos.shape
    F = obstacle_faces.shape[1]
    shards = N_CORES // B            # row shards per batch
    rows = N // shards               # rows per core
    row_tiles = rows // 128

    in_maps = []
    for core in range(N_CORES):
        b, r = divmod(core, shards)
        face_curr = _face_tables(
            np.asarray(obstacle_curr_pos[b], dtype=np.float32),
            np.asarray(obstacle_faces[b]),
        )
        x = np.asarray(cloth_curr_pos[b, r * rows:(r + 1) * rows], dtype=np.float32)
        xt4, crhs = _prep_core_inputs(x, face_curr, row_tiles)
        in_maps.append({"xt4": xt4, "crhs": crhs})

    res = _run_device(in_maps, row_tiles, F, trace=trace)

    nn_idx = np.empty((B, N), dtype=np.int64)
    for core in range(N_CORES):
        b, r = divmod(core, shards)
        cidx = res.results[core]["cidx"].astype(np.int64)   # [128, row_tiles]
        widx = res.results[core]["widx"].astype(np.int64)
        local = cidx * MINI + (widx % MINI)                 # [128, row_tiles]
        # global row = r*rows + t*128 + p
        nn_idx[b, r * rows:(r + 1) * rows] = local.T.reshape(-1)
    return nn_idx, res


def _loss_epilogue(cloth_next_pos, obstacle_next_pos, obstacle_faces, nn_idx, weight):
    B, N, _ = cloth_next_pos.shape
    per_vert_sum = np.zeros(N, dtype=np.float32)
    loss_sum = np.float32(0.0)
    for b in range(B):
        faces = np.asarray(obstacle_faces[b])
        onp = np.asarray(obstacle_next_pos[b], dtype=np.float32)
        v = onp[faces]                                       # [F, 3, 3]
        face_next = v.mean(axis=1, dtype=np.float32)         # [F, 3]
        n = np.cross(v[:, 1] - v[:, 0], v[:, 2] - v[:, 0]).astype(np.float32)
        norm = np.sqrt(np.sum(n * n, axis=1, dtype=np.float32)).astype(np.float32)
        fn = n / (norm[:, None] + np.float32(1e-12))
        idx = nn_idx[b]
        nn_points = face_next[idx]                           # [N, 3]
        nn_normals = fn[idx]                                 # [N, 3]
        nxt = np.asarray(cloth_next_pos[b], dtype=np.float32)
        distance = np.sum((nxt - nn_points) * nn_normals, axis=1, dtype=np.float32)
        interp = np.maximum(np.float32(EPS) - distance, np.float32(0.0)) ** 3
        per_vert_sum += interp
        loss_sum += interp.sum(dtype=np.float32)
    w = np.float32(weight)
    loss = np.float32(loss_sum / np.float32(B) * w)
    per_vert = (per_vert_sum / np.float32(B) * w).astype(np.float32)
    return loss, per_vert


def _weight(iter_num) -> float:
    it = max(int(iter_num) - START_RAMPUP, 0)
    progress = min(it / N_RAMPUP, 1.0)
    return WEIGHT_START + (WEIGHT_MAX - WEIGHT_START) * progress


def kernel(cloth_curr_pos, cloth_next_pos, obstacle_curr_pos, obstacle_next_pos,
           obstacle_faces, iter_num):
    cloth_curr_pos = np.asarray(cloth_curr_pos)
    cloth_next_pos = np.asarray(cloth_next_pos)
    obstacle_curr_pos = np.asarray(obstacle_curr_pos)
    obstacle_next_pos = np.asarray(obstacle_next_pos)
    obstacle_faces = np.asarray(obstacle_faces)

    nn_idx, _ = compute_nn_idx(cloth_curr_pos, obstacle_curr_pos, obstacle_faces)
    return _loss_epilogue(
        cloth_next_pos, obstacle_next_pos, obstacle_faces, nn_idx,
        _weight(iter_num),
    )
